# revision 3
# baseline (speedup 1.0000x reference)
"""Llama-style GQA attention (B=1, S=2048, HID=4096, 32 Q heads / 8 KV heads),
tensor-parallel over 8 NeuronCores.

Sharding: core c owns Q heads 4c..4c+3 (512 output features of Wq), KV head c
(128 features of Wk/Wv), and the matching 512 input rows of Wo.T.  Each core
computes its partial o_proj contribution over the full sequence; the unshard
step sums the 8 partials (the all-reduce of the TP layout, done host-side).

On-device per core: x is transposed on the TensorEngine (hid is the matmul
contraction dim), then QKV projections + RoPE + causal softmax attention +
o_proj run as bf16 matmuls with fp32 PSUM accumulation; softmax is fp32.
"""
import math
import sys
from contextlib import ExitStack

sys.path.insert(0, "/opt/trn_rl_repo")

import numpy as np

import concourse.bacc as bacc
import concourse.bass as bass
import concourse.mybir as mybir
import concourse.tile as tile
from concourse.bass_utils import run_bass_kernel_spmd
from concourse.masks import make_identity

B, S, HID = 1, 2048, 4096
NH, NKV = 32, 8
HD = 128
N_CORES = 8
HPC = NH // N_CORES          # 4 Q heads per core
QD = HPC * HD                # 512
P = 128
SBN = S // P                 # 16 seq blocks
HC = HID // P                # 32 hid chunks
ROPE_THETA = 10000.0
SCALE = 1.0 / math.sqrt(HD)
MASK_VAL = -1.0e5

F32 = mybir.dt.float32
BF16 = mybir.dt.bfloat16

_CACHE = {}


def _build_nc():
    nc = bacc.Bacc(None, target_bir_lowering=False, debug=False,
                   num_devices=N_CORES)
    x_d = nc.dram_tensor("x", (S, HID), F32, kind="ExternalInput").ap()
    wqt_d = nc.dram_tensor("wqt", (HID, QD), F32, kind="ExternalInput").ap()
    wkt_d = nc.dram_tensor("wkt", (HID, HD), F32, kind="ExternalInput").ap()
    wvt_d = nc.dram_tensor("wvt", (HID, HD), F32, kind="ExternalInput").ap()
    wot_d = nc.dram_tensor("wot", (QD, HID), F32, kind="ExternalInput").ap()
    cos_d = nc.dram_tensor("cosn", (S, HD), F32, kind="ExternalInput").ap()
    sin_d = nc.dram_tensor("sinn", (S, HD), F32, kind="ExternalInput").ap()
    o_d = nc.dram_tensor("o_part", (S, HID), F32, kind="ExternalOutput").ap()

    with tile.TileContext(nc) as tc, ExitStack() as ctx:
        singles = ctx.enter_context(tc.tile_pool(name="singles", bufs=1))

        # ---- resident tensors -------------------------------------------
        ident = singles.tile([P, P], BF16)
        make_identity(nc, ident)

        # additive causal masks for the diagonal 512-wide panel, r = qb % 4:
        # mask[i, j] = 0 if i + 128*r >= j else MASK_VAL
        masks = singles.tile([P, 4, 512], BF16)
        nc.gpsimd.memset(masks, 0.0)
        for r in range(4):
            nc.gpsimd.affine_select(
                out=masks[:, r, :], in_=masks[:, r, :],
                compare_op=mybir.AluOpType.is_ge, fill=MASK_VAL,
                base=r * P, pattern=[[-1, 512]], channel_multiplier=1,
            )

        cosn = singles.tile([P, SBN, HD], BF16)
        sinn = singles.tile([P, SBN, HD], BF16)

        wq16 = singles.tile([P, HC, QD], BF16)       # [hidc, q feats]
        wkv16 = singles.tile([P, HC, 2 * HD], BF16)  # [k | v]
        wo16 = singles.tile([P, HPC, HID], BF16)

        v_nat = singles.tile([P, SBN, HD], BF16)     # v natural [seq, hd]
        qt = singles.tile([P, HPC, S], BF16)         # q^T per head [hd, seq]
        kt = singles.tile([P, S], BF16)              # k^T [hd, seq]
        att = singles.tile([P, HPC, SBN, P], BF16)   # attn^T [hd, (h, seq)]

        # ---- weight + rope-table load (pool closes before the big phases)
        with tc.tile_pool(name="wload", bufs=2) as wload:
            ctmp = wload.tile([P, SBN, HD], F32, tag="ct")
            nc.sync.dma_start(out=ctmp, in_=cos_d.rearrange("(sb p) d -> p sb d", p=P))
            nc.vector.tensor_copy(cosn, ctmp)
            stmp = wload.tile([P, SBN, HD], F32, tag="ct")
            nc.sync.dma_start(out=stmp, in_=sin_d.rearrange("(sb p) d -> p sb d", p=P))
            nc.vector.tensor_copy(sinn, stmp)
            for g in range(16):                      # 2 hid chunks per DMA
                wtmp = wload.tile([P, 2, QD], F32, tag="wq")
                nc.sync.dma_start(
                    out=wtmp,
                    in_=wqt_d[g * 256:(g + 1) * 256, :].rearrange(
                        "(c p) m -> p c m", p=P))
                nc.vector.tensor_copy(wq16[:, g * 2:(g + 1) * 2, :], wtmp)
            for g in range(4):                       # 8 hid chunks per DMA
                wtmp = wload.tile([P, 8, HD], F32, tag="wk")
                nc.sync.dma_start(
                    out=wtmp,
                    in_=wkt_d[g * 1024:(g + 1) * 1024, :].rearrange(
                        "(c p) m -> p c m", p=P))
                nc.vector.tensor_copy(wkv16[:, g * 8:(g + 1) * 8, 0:HD], wtmp)
                wtmp2 = wload.tile([P, 8, HD], F32, tag="wk")
                nc.sync.dma_start(
                    out=wtmp2,
                    in_=wvt_d[g * 1024:(g + 1) * 1024, :].rearrange(
                        "(c p) m -> p c m", p=P))
                nc.vector.tensor_copy(wkv16[:, g * 8:(g + 1) * 8, HD:2 * HD], wtmp2)
            for h in range(HPC):
                for hf in range(2):
                    wtmp3 = wload.tile([P, HID // 2], F32, tag="wo")
                    nc.sync.dma_start(
                        out=wtmp3,
                        in_=wot_d[h * P:(h + 1) * P,
                                  hf * (HID // 2):(hf + 1) * (HID // 2)])
                    nc.vector.tensor_copy(
                        wo16[:, h, hf * (HID // 2):(hf + 1) * (HID // 2)], wtmp3)

        xload = ctx.enter_context(tc.tile_pool(name="xload", bufs=2))
        work = ctx.enter_context(tc.tile_pool(name="work", bufs=2))
        xtp = ctx.enter_context(tc.tile_pool(name="xtp", bufs=4))
        ppool = ctx.enter_context(tc.tile_pool(name="ppool", bufs=2))
        ptp = ctx.enter_context(tc.tile_pool(name="ptp", bufs=4))
        opool = ctx.enter_context(tc.tile_pool(name="opool", bufs=2))
        # single PSUM pool, tags share the 8 banks:
        # tr(2) + proj(2) + projB(1) + sc(2) + at(1) = 8 banks
        psum = ctx.enter_context(tc.tile_pool(name="psum", bufs=1, space="PSUM"))

        # ---- phase 1: x transpose + QKV projections + RoPE --------------
        for sb in range(SBN):
            x16 = work.tile([P, HID], BF16, tag="x16")
            for hf in range(2):
                x_f32 = xload.tile([P, HID // 2], F32, tag="xf")
                nc.sync.dma_start(
                    out=x_f32,
                    in_=x_d[sb * P:(sb + 1) * P,
                            hf * (HID // 2):(hf + 1) * (HID // 2)])
                nc.vector.tensor_copy(
                    x16[:, hf * (HID // 2):(hf + 1) * (HID // 2)], x_f32)
            psA = psum.tile([P, QD], F32, tag="proj", bufs=2)
            psB = psum.tile([P, 2 * HD], F32, tag="projB", bufs=1)
            for hc in range(HC):
                tp = psum.tile([P, P], BF16, tag="tr", bufs=2)
                nc.tensor.transpose(tp, x16[:, hc * P:(hc + 1) * P], ident)
                xtc = xtp.tile([P, P], BF16, tag="xtc")
                nc.vector.tensor_copy(xtc, tp)
                nc.tensor.matmul(psA, xtc, wq16[:, hc, :],
                                 start=(hc == 0), stop=(hc == HC - 1))
                nc.tensor.matmul(psB, xtc, wkv16[:, hc, :],
                                 start=(hc == 0), stop=(hc == HC - 1))
            nc.vector.tensor_copy(v_nat[:, sb, :], psB[:, HD:2 * HD])
            # RoPE on q (4 heads) and k
            q_ro = work.tile([P, QD], BF16, tag="qro")
            k_ro = work.tile([P, HD], BF16, tag="kro")
            for h in range(HPC + 1):
                src = psB[:, 0:HD] if h == HPC else psA[:, h * HD:(h + 1) * HD]
                dst = k_ro if h == HPC else q_ro[:, h * HD:(h + 1) * HD]
                m1 = work.tile([P, HD], F32, tag="m1")
                m2 = work.tile([P, HD], F32, tag="m2")
                nc.vector.tensor_mul(m1, src, cosn[:, sb, :])
                nc.vector.tensor_mul(m2[:, 0:64], src[:, 64:128], sinn[:, sb, 0:64])
                nc.vector.tensor_mul(m2[:, 64:128], src[:, 0:64], sinn[:, sb, 64:128])
                nc.vector.tensor_sub(dst[:, 0:64], m1[:, 0:64], m2[:, 0:64])
                nc.vector.tensor_add(dst[:, 64:128], m1[:, 64:128], m2[:, 64:128])
            for h in range(HPC):
                tp = psum.tile([P, P], BF16, tag="tr", bufs=2)
                nc.tensor.transpose(tp, q_ro[:, h * HD:(h + 1) * HD], ident)
                nc.vector.tensor_copy(qt[:, h, sb * P:(sb + 1) * P], tp)
            tp = psum.tile([P, P], BF16, tag="tr", bufs=2)
            nc.tensor.transpose(tp, k_ro, ident)
            nc.vector.tensor_copy(kt[:, sb * P:(sb + 1) * P], tp)

        # ---- phase 2: causal attention ----------------------------------
        for h in range(HPC):
            for qb in range(SBN):
                npan = qb // 4 + 1
                rr = qb % 4
                l_parts = ppool.tile([P, npan], F32, tag="lp")
                ps_a = psum.tile([P, P], F32, tag="at", bufs=1)
                first = True
                for pan in range(npan):
                    diag = pan == npan - 1
                    ps_s = psum.tile([P, 512], F32, tag="sc", bufs=2)
                    nc.tensor.matmul(ps_s, qt[:, h, qb * P:(qb + 1) * P],
                                     kt[:, pan * 512:(pan + 1) * 512],
                                     start=True, stop=True)
                    if diag:
                        nc.vector.tensor_add(ps_s, ps_s, masks[:, rr, :])
                    p_sb = ppool.tile([P, 512], BF16, tag="p")
                    nc.scalar.activation(p_sb, ps_s,
                                         mybir.ActivationFunctionType.Exp,
                                         scale=SCALE,
                                         accum_out=l_parts[:, pan:pan + 1])
                    n_kt = (rr + 1) if diag else 4
                    for kti in range(n_kt):
                        tp = psum.tile([P, P], BF16, tag="tr", bufs=2)
                        nc.tensor.transpose(tp, p_sb[:, kti * P:(kti + 1) * P],
                                            ident)
                        pt_sb = ptp.tile([P, P], BF16, tag="pt")
                        nc.vector.tensor_copy(pt_sb, tp)
                        nc.tensor.matmul(ps_a, pt_sb, v_nat[:, pan * 4 + kti, :],
                                         start=first,
                                         stop=(diag and kti == n_kt - 1))
                        first = False
                l_sum = ppool.tile([P, 1], F32, tag="ls")
                nc.vector.tensor_reduce(l_sum, l_parts, axis=mybir.AxisListType.X,
                                        op=mybir.AluOpType.add)
                rl = ppool.tile([P, 1], F32, tag="rl")
                nc.vector.reciprocal(rl, l_sum)
                a_sc = ppool.tile([P, P], BF16, tag="asc")
                nc.vector.tensor_scalar_mul(a_sc, ps_a, rl)
                tp = psum.tile([P, P], BF16, tag="tr", bufs=2)
                nc.tensor.transpose(tp, a_sc, ident)
                nc.vector.tensor_copy(att[:, h, qb, :], tp)

        # ---- phase 3: o_proj partial ------------------------------------
        for sb in range(SBN):
            for half in range(2):
                o_row = opool.tile([P, HID // 2], F32, tag="orow")
                for pp in range(4):
                    pan = half * 4 + pp
                    pso = psum.tile([P, 512], F32, tag="proj", bufs=2)
                    for h in range(HPC):
                        nc.tensor.matmul(pso, att[:, h, sb, :],
                                         wo16[:, h, pan * 512:(pan + 1) * 512],
                                         start=(h == 0), stop=(h == HPC - 1))
                    nc.scalar.copy(o_row[:, pp * 512:(pp + 1) * 512], pso)
                nc.sync.dma_start(
                    out=o_d[sb * P:(sb + 1) * P,
                            half * (HID // 2):(half + 1) * (HID // 2)],
                    in_=o_row)

    nc.compile()
    return nc


def kernel(x, Wq, Wk, Wv, Wo, position_ids):
    x = np.asarray(x, dtype=np.float32)
    Wq = np.asarray(Wq, dtype=np.float32)
    Wk = np.asarray(Wk, dtype=np.float32)
    Wv = np.asarray(Wv, dtype=np.float32)
    Wo = np.asarray(Wo, dtype=np.float32)
    pos = np.asarray(position_ids)

    if "nc" not in _CACHE:
        _CACHE["nc"] = _build_nc()
    nc = _CACHE["nc"]

    x2d = np.ascontiguousarray(x.reshape(S, HID))
    # rope buffers (replicated), computed from position_ids as in the reference
    inv_freq = 1.0 / (ROPE_THETA ** (np.arange(0, HD, 2, dtype=np.float32) / HD))
    t = pos[0].astype(np.float32)
    freqs = t[:, None] * inv_freq[None, :]
    emb = np.concatenate([freqs, freqs], axis=-1)
    cosn = np.cos(emb).astype(np.float32)
    sinn = np.sin(emb).astype(np.float32)

    in_maps = []
    for c in range(N_CORES):
        in_maps.append({
            "x": x2d,
            "wqt": np.ascontiguousarray(Wq[c * QD:(c + 1) * QD, :].T),
            "wkt": np.ascontiguousarray(Wk[c * HD:(c + 1) * HD, :].T),
            "wvt": np.ascontiguousarray(Wv[c * HD:(c + 1) * HD, :].T),
            "wot": np.ascontiguousarray(Wo[:, c * QD:(c + 1) * QD].T),
            "cosn": cosn,
            "sinn": sinn,
        })

    res = run_bass_kernel_spmd(nc, in_maps, list(range(N_CORES)))
    _CACHE["last_res"] = res
    out = np.zeros((S, HID), dtype=np.float32)
    for c in range(N_CORES):
        out += res.results[c]["o_part"]
    return out.reshape(B, S, HID)
